# revision 1
# baseline (speedup 1.0000x reference)
"""Trainium2 Bass kernel for nn_AutoencoderHybrid (12-qubit QAE hybrid).

Math: the circuit measures Z on wires 0..3 only. The CNOT chain (i -> i+1)
propagates information forward only, so in the Heisenberg picture each
observable Z_w pulled back through the 2-layer circuit is supported on wires
0..w+1 (at most wires 0..4). With the product input state
|psi_b> = kron_j [cos(x_j/2), -i sin(x_j/2)], the diagonal phase factors
D = diag((-i)^popcount) fold into the observable, leaving a REAL quadratic
form on a 32-dim real product vector:

    latent_w(b) = r_b^T S_w r_b,  r_b = kron_{j=0..4} [cos(x_j/2), sin(x_j/2)]
    S_w = Re(D^H V^H Z_w V D)     (V = truncated 5-qubit circuit unitary)

followed by the decoder MLP: out = relu(lat @ W1.T + b1) @ W2.T + b2.

S_w / MLP weights are tiny (depend only on q_params etc.) and are prepared on
the host; all batch-dim work (B = 8192) runs on 8 NeuronCores, data parallel,
1024 rows per core laid out as b = 8p + q (p partition, q free-dim group).

Device pipeline per core (single macro pass, fully unrolled Tile kernel):
  1. one contiguous 48KB x DMA; quarter-angle Sin activations (arg range
     of the ACT Sin table is [-pi, pi], so sin/cos(x/2) are assembled from
     s4 = sin(x/4), c4 = cos(x/4): cos(x/2)/2 = c4^2 - 1/2, sin(x/2)/2 =
     s4*c4; the uniform 1/32 scale of r is folded into S on the host)
  2. kron tree build of r per half (4 groups): A = u0(x)u1, C = u3(x)u4,
     B = A(x)u2, r = B(x)C — one tensor_tensor op per node via step-0
     broadcast access patterns; half 0 on DVE, half 1 on GPSIMD in parallel
  3. PE transpose of r + copy to SBUF as float32r; block-diagonal f32r
     matmul Y = rT.T @ Mblk (4 groups x 4 observables at once, split into
     2x256-col matmuls to start as soon as each mblk DMA half completes)
  4. lat = rowsum(r * Y) per half on DVE (multiply + strided reduce)
  5. MLP runs in transposed space: hT = relu(W1T_blk.T @ latT + b1) with
     b1/b2 added via early K=1 ones-row PSUM-prefill matmuls, then
     y = hT.T @ W2T_blk; per-half chains overlap across PE/DVE/ACT
  6. one contiguous 48KB y DMA out.
"""
import math
import numpy as np

N5 = 5
NLAYERS = 2
LATENT = 4
B = 8192
NCORES = 8
BLOC = B // NCORES  # 1024

# ----------------------------------------------------------------------------
# Host-side constant construction (pure numpy)
# ----------------------------------------------------------------------------


def _rot(phi, theta, omega):
    c, s = np.cos(theta / 2), np.sin(theta / 2)
    ep = np.exp(-0.5j * (phi + omega))
    em = np.exp(-0.5j * (phi - omega))
    return np.array([[ep * c, -np.conj(em) * s], [em * s, np.conj(ep) * c]],
                    dtype=np.complex128)


def _build_S(q_params):
    """(4, 32, 32) real symmetric: latent_w = r^T S_w r."""
    qp = np.asarray(q_params, np.float64)
    dim = 2 ** N5
    eye2 = np.eye(2)

    def kron_at(U, wire):
        M = np.array([[1.0]])
        for j in range(N5):
            M = np.kron(M, U if j == wire else eye2)
        return M

    def cnot_mat(c, t):
        M = np.zeros((dim, dim))
        for z in range(dim):
            bits = [(z >> (N5 - 1 - j)) & 1 for j in range(N5)]
            if bits[c] == 1:
                bits[t] ^= 1
            z2 = 0
            for b in bits:
                z2 = (z2 << 1) | b
            M[z2, z] = 1.0
        return M

    V = np.eye(dim, dtype=np.complex128)
    for l in range(NLAYERS):
        for i in range(N5):
            V = kron_at(_rot(*qp[l, i]), i) @ V
        for i in range(N5 - 1):
            V = cnot_mat(i, i + 1) @ V

    pc = np.array([bin(z).count("1") for z in range(dim)])
    D = np.diag((-1j) ** pc)
    VD = V @ D
    Ss = []
    for w in range(LATENT):
        zdiag = np.array([1.0 if ((z >> (N5 - 1 - w)) & 1) == 0 else -1.0
                          for z in range(dim)])
        O = VD.conj().T @ (zdiag[:, None] * VD)
        Ss.append(np.real(O))
    return np.stack(Ss)


def _host_consts(q_params, W1, b1, W2, b2):
    S = _build_S(q_params)                       # (4, 32, 32)
    # device computes r' with cos/2 and sin/2 slots: r' = r/32.
    # compensate: S' = 1024 * S  (uniform; exact power of two)
    S = S * 1024.0
    mmat = np.concatenate([S[w] for w in range(4)], axis=1)  # (32, 128)
    mblk = np.zeros((128, 512), np.float32)
    for g in range(4):
        mblk[32 * g:32 * g + 32, 128 * g:128 * g + 128] = mmat
    W1 = np.asarray(W1, np.float64)
    b1 = np.asarray(b1, np.float64)
    W2 = np.asarray(W2, np.float64)
    b2 = np.asarray(b2, np.float64)
    w1tblk = np.zeros((16, 128), np.float32)
    for a in range(4):
        w1tblk[4 * a:4 * a + 4, 32 * a:32 * a + 32] = W1.T
    b1col = np.tile(b1, 4)[None, :].astype(np.float32)
    w2blk = np.zeros((128, 48), np.float32)
    for g in range(4):
        w2blk[32 * g:32 * g + 32, 12 * g:12 * g + 12] = W2.T
    b2row = np.tile(b2, 4)[None, :].astype(np.float32)
    return dict(mblk=mblk, w1tblk=w1tblk, b1col=b1col, w2blk=w2blk,
                b2row=b2row)



# ----------------------------------------------------------------------------
# Device kernel body (Bass/Tile)
# ----------------------------------------------------------------------------


def _build_body(ctx, tc, x, mblk, w1tblk, b1col, w2blk, b2row, y):
    import concourse.bass as bass
    from concourse import mybir
    nc = tc.nc
    f32 = mybir.dt.float32
    f32r = mybir.dt.float32r
    AF = mybir.ActivationFunctionType

    def fv(t, col, dims):
        """View of tile t at free-offset col with custom free dims."""
        return bass.AP(tensor=t.tensor, offset=t.offset + col,
                       ap=[list(t.ap[0])] + [list(d) for d in dims])

    consts = ctx.enter_context(tc.tile_pool(name="consts", bufs=1))
    sb = ctx.enter_context(tc.tile_pool(name="sb", bufs=1))
    sb2 = ctx.enter_context(tc.tile_pool(name="sb2", bufs=2))
    ps = ctx.enter_context(tc.tile_pool(name="ps", bufs=1, space="PSUM"))
    ps2 = ctx.enter_context(tc.tile_pool(name="ps2", bufs=2, space="PSUM"))

    # ---- x load first: everything downstream waits on it (SP queue)
    x_s = sb.tile([128, 96], f32)
    xa = bass.AP(tensor=x.tensor, offset=0, ap=[[96, 128], [1, 96]])
    nc.sync.dma_start(x_s[:, :], xa)

    # ---- constants on the second HWDGE queue (Activation engine)
    mblk_s = consts.tile([128, 512], f32r)
    mblk_r = mblk.bitcast(f32r)
    nc.sync.dma_start(mblk_s[:, 0:256],
                      bass.AP(tensor=mblk_r.tensor, offset=0,
                              ap=[[512, 128], [1, 256]]))
    nc.sync.dma_start(mblk_s[:, 256:512],
                      bass.AP(tensor=mblk_r.tensor, offset=256,
                              ap=[[512, 128], [1, 256]]))
    w1t_s = consts.tile([16, 128], f32r)
    nc.sync.dma_start(w1t_s[:, :], w1tblk.bitcast(f32r))
    w2_s = consts.tile([128, 48], f32r)
    nc.sync.dma_start(w2_s[:, :], w2blk.bitcast(f32r))
    b1c_s = consts.tile([1, 128], f32)
    nc.sync.dma_start(b1c_s[:, :], b1col)
    b2_s = consts.tile([1, 48], f32)
    nc.sync.dma_start(b2_s[:, :], b2row)
    ones1 = consts.tile([1, 128], f32)
    nc.vector.memset(ones1[:, :], 1.0)
    bias_c = consts.tile([128, 1], f32)
    nc.vector.memset(bias_c[:, :], math.pi / 2)
    bias_s = consts.tile([128, 1], f32)
    nc.vector.memset(bias_s[:, :], 0.0)
    ones128 = consts.tile([128, 128], f32)
    nc.vector.memset(ones128[:, :], 1.0)
    ident = consts.tile([128, 128], f32)
    nc.gpsimd.affine_select(out=ident[:, :], in_=ones128[:, :],
                            pattern=[[1, 128]],
                            compare_op=mybir.AluOpType.is_equal, fill=0.0,
                            base=0, channel_multiplier=-1)
    # warm the ACT Sin table while DMAs run (cold table load ~1.4us)
    warm = consts.tile([128, 1], f32)
    nc.scalar.activation(warm[:, :], bias_s[:, 0:1], AF.Sin,
                         bias=bias_s[:, 0:1], scale=1.0)

    # ---- quarter angle: s4 = sin(x/4), c4 = cos(x/4) = sin(x/4 + pi/2)
    # cs[p, 10c + j]     = cos(x/2) = c4^2 - s4^2
    # cs[p, 10c + 5 + j] = sin(x/2)/2 = s4*c4  (x2 folded into S on host)
    # head split per half: h=0 chain on DVE, h=1 chain on GPSIMD (parallel)
    sc4 = sb.tile([128, 80], f32)
    for h in (0, 1):
        xin = fv(x_s, 48 * h, [[12, 4], [1, 5]])
        nc.scalar.activation(fv(sc4, 40 * h, [[10, 4], [1, 5]]), xin, AF.Sin,
                             bias=bias_s[:, 0:1], scale=0.25)
        nc.scalar.activation(fv(sc4, 40 * h + 5, [[10, 4], [1, 5]]), xin,
                             AF.Sin, bias=bias_c[:, 0:1], scale=0.25)

    csh, rh = [], []
    for h in (0, 1):
        E = nc.vector if h == 0 else nc.gpsimd
        sqt = sb.tile([128, 20], f32, name=f"sq{h}")
        cst = sb.tile([128, 40], f32, name=f"cs{h}")
        # cs_c = cos(x/2)/2 = c4^2 - 1/2 ; cs_s = sin(x/2)/2 = s4*c4
        E.tensor_mul(fv(sqt, 0, [[5, 4], [1, 5]]),
                     fv(sc4, 40 * h + 5, [[10, 4], [1, 5]]),
                     fv(sc4, 40 * h + 5, [[10, 4], [1, 5]]))
        E.tensor_mul(fv(cst, 5, [[10, 4], [1, 5]]),
                     fv(sc4, 40 * h, [[10, 4], [1, 5]]),
                     fv(sc4, 40 * h + 5, [[10, 4], [1, 5]]))
        E.tensor_scalar_sub(fv(cst, 0, [[10, 4], [1, 5]]),
                            fv(sqt, 0, [[5, 4], [1, 5]]), 0.5)
        # kron tree: A = u0(x)u1, C = u3(x)u4, B = A(x)u2, r = B(x)C
        # single-op krons via step-0 broadcast dims on both operands
        A = sb.tile([128, 16], f32, name=f"A{h}")
        C = sb.tile([128, 16], f32, name=f"C{h}")
        Bt = sb.tile([128, 32], f32, name=f"B{h}")
        rt = sb.tile([128, 128], f32, name=f"r{h}")
        # A[4c + 2 z0 + z1] = u0[z0] * u1[z1]
        E.tensor_mul(fv(A, 0, [[4, 4], [2, 2], [1, 2]]),
                     fv(cst, 1, [[10, 4], [0, 2], [5, 2]]),
                     fv(cst, 0, [[10, 4], [5, 2], [0, 2]]))
        # C[4c + 2 z3 + z4] = u3[z3] * u4[z4]
        E.tensor_mul(fv(C, 0, [[4, 4], [2, 2], [1, 2]]),
                     fv(cst, 4, [[10, 4], [0, 2], [5, 2]]),
                     fv(cst, 3, [[10, 4], [5, 2], [0, 2]]))
        # B[8c + 4 z0 + 2 z1 + z2] = A[4c + 2 z0 + z1] * u2[z2]
        E.tensor_mul(fv(Bt, 0, [[8, 4], [2, 4], [1, 2]]),
                     fv(cst, 2, [[10, 4], [0, 4], [5, 2]]),
                     fv(A, 0, [[4, 4], [1, 4], [0, 2]]))
        # r[32c + 4 beta + gamma] = B[8c + beta] * C[4c + gamma]
        E.tensor_mul(fv(rt, 0, [[32, 4], [4, 8], [1, 4]]),
                     fv(Bt, 0, [[8, 4], [1, 8], [0, 4]]),
                     fv(C, 0, [[4, 4], [0, 8], [1, 4]]))
        csh.append(cst)
        rh.append(rt)

    # ---- quadratic form per half (4 groups): lat_all[p, 4q + w]
    lat_all = sb.tile([128, 32], f32)
    ymm_insts = []
    Pm = sb.tile([128, 1024], f32)
    for h in (0, 1):
        rT4_p = ps2.tile([128, 128], f32, tag="t128", bufs=2)
        nc.tensor.transpose(rT4_p[:, :], rh[h][:, :], ident[:, :])
        rT4_s = sb2.tile([128, 128], f32r, tag="rT4s")
        if h == 0:
            nc.vector.tensor_copy(rT4_s[:, :], rT4_p[:, :])
        else:
            nc.scalar.copy(rT4_s[:, :], rT4_p[:, :])
        Y4_p = ps2.tile([128, 512], f32, tag="Y4")
        for half in (0, 1):
            ymm_insts.append(nc.tensor.matmul(
                Y4_p[:, 256 * half:256 * half + 256], lhsT=rT4_s[:, :],
                rhs=mblk_s[:, 256 * half:256 * half + 256],
                start=True, stop=True))
        nc.vector.tensor_mul(fv(Pm, 512 * h, [[128, 4], [32, 4], [1, 32]]),
                             fv(Y4_p, 0, [[128, 4], [32, 4], [1, 32]]),
                             fv(rh[h], 0, [[32, 4], [0, 4], [1, 32]]))

    for h in (0, 1):
        nc.vector.reduce_sum(fv(lat_all, 16 * h, [[4, 4], [1, 4]]),
                             fv(Pm, 512 * h, [[128, 4], [32, 4], [1, 32]]),
                             axis=mybir.AxisListType.X)

    # ---- PSUM bias prefills (emitted after quadform so they don't block
    # the r transposes in PE program order; they only need b1/b2 consts)
    from concourse.bass import _add_dep_helper
    hT_ps, y4_ps = [], []
    for h in (0, 1):
        hT_p = ps2.tile([128, 128], f32, tag="hTp")
        i1 = nc.tensor.matmul(hT_p[:, :], lhsT=b1c_s[:, :], rhs=ones1[:, :],
                              start=True, stop=False)
        hT_ps.append(hT_p)
        y4_p = ps2.tile([128, 48], f32, tag="y4")
        i2 = nc.tensor.matmul(y4_p[:, :], lhsT=ones1[:, :], rhs=b2_s[:, :],
                              start=True, stop=False)
        y4_ps.append(y4_p)
        # keep prefills behind the quadform matmuls in PE order: they wait
        # on b1/b2 DMA completion and would otherwise stall the PE queue
        for i in (i1, i2):
            _add_dep_helper(i.ins, ymm_insts[-1].ins, sync=False,
                            reason="prefill after Ymm")

    # ---- MLP in transposed space: hT[32a+j, b'] = relu(W1 latT + b1)
    # per-half latT transpose/copy so h0's MLP starts right after its reduce
    y_s = sb.tile([128, 96], f32)
    for h in (0, 1):
        latT_p = ps2.tile([16, 128], f32, tag="t128", bufs=2)
        nc.tensor.transpose(latT_p[:, :], lat_all[:, 16 * h:16 * h + 16],
                            ident[:, :])
        latT_s = sb2.tile([16, 128], f32r, tag="latTs")
        nc.vector.tensor_copy(latT_s[:, :], latT_p[:, :])
        nc.tensor.matmul(hT_ps[h][:, :], lhsT=w1t_s[:, :],
                         rhs=latT_s[:, :],
                         start=False, stop=True)
        hT_s = sb2.tile([128, 128], f32r, tag="hTs")
        if h == 0:
            nc.vector.tensor_scalar_max(hT_s[:, :], hT_ps[h][:, :], 0.0)
        else:
            nc.scalar.activation(hT_s[:, :], hT_ps[h][:, :], AF.Relu)
        nc.tensor.matmul(y4_ps[h][:, :], lhsT=hT_s[:, :], rhs=w2_s[:, :],
                         start=False, stop=True)
        if h == 0:
            nc.scalar.copy(fv(y_s, 48 * h, [[12, 4], [1, 12]]),
                           fv(y4_ps[h], 0, [[12, 4], [1, 12]]))
        else:
            nc.vector.tensor_copy(fv(y_s, 48 * h, [[12, 4], [1, 12]]),
                                  fv(y4_ps[h], 0, [[12, 4], [1, 12]]))
    ya = bass.AP(tensor=y.tensor, offset=0, ap=[[96, 128], [1, 96]])
    nc.sync.dma_start(ya, y_s[:, :])



_NC_CACHE = {}


def _get_nc():
    if "nc" in _NC_CACHE:
        return _NC_CACHE["nc"]
    from contextlib import ExitStack
    import concourse.bacc as bacc
    import concourse.tile as tile
    from concourse import mybir
    f32 = mybir.dt.float32
    nc = bacc.Bacc("TRN2", target_bir_lowering=False, debug=False)
    x = nc.dram_tensor("x", [BLOC, 12], f32, kind="ExternalInput").ap()
    mblk = nc.dram_tensor("mblk", [128, 512], f32, kind="ExternalInput").ap()
    w1tblk = nc.dram_tensor("w1tblk", [16, 128], f32, kind="ExternalInput").ap()
    b1col = nc.dram_tensor("b1col", [1, 128], f32, kind="ExternalInput").ap()
    w2blk = nc.dram_tensor("w2blk", [128, 48], f32, kind="ExternalInput").ap()
    b2row = nc.dram_tensor("b2row", [1, 48], f32, kind="ExternalInput").ap()
    y = nc.dram_tensor("y", [BLOC, 12], f32, kind="ExternalOutput").ap()
    with tile.TileContext(nc) as tc:
        with ExitStack() as ctx:
            _build_body(ctx, tc, x, mblk, w1tblk, b1col, w2blk, b2row, y)
    nc.compile()
    _NC_CACHE["nc"] = nc
    return nc


def _run(inputs_np, consts, trace=False):
    from concourse.bass_utils import run_bass_kernel_spmd
    nc = _get_nc()
    x = np.ascontiguousarray(np.asarray(inputs_np, np.float32))
    in_maps = []
    for c in range(NCORES):
        m = {"x": np.ascontiguousarray(x[BLOC * c:BLOC * (c + 1)])}
        m.update(consts)
        in_maps.append(m)
    res = run_bass_kernel_spmd(nc, in_maps, core_ids=list(range(NCORES)),
                               trace=trace)
    out = np.concatenate([r["y"] for r in res.results], axis=0)
    return out.astype(np.float32), res


def kernel(inputs, q_params, W1, b1, W2, b2):
    consts = _host_consts(q_params, W1, b1, W2, b2)
    out, _ = _run(inputs, consts, trace=False)
    return out

